# revision 4
# baseline (speedup 1.0000x reference)
"""GCN layer (SpMM + Linear) on 8 Trainium2 NeuronCores — v8.

out[i] = (sum_{e: row[e]==i} val[e] * X[col[e]]) @ W.T + b

v8 = v7 (4-SWDGE-queue gathers, batched one-hot on DVE + val-multiply
split DVE/Act, 128-dest supers, on-chip transpose + linear) plus
load-balanced destination binning: instead of consecutive 128-row
supers, each core's 12500 dest rows are LPT-packed into 99 supers of
<=128 dests so that every (super, chunk) edge-group count stays <=1024
on every core. Group capacities then drop from ~1152 (max-over-cores
+ 128-rounding) to a uniform 1024: ~10% fewer gather descriptors and
~10% less gather DMA traffic. The device writes super-major rows; the
host inverse-permutes them into the natural row order.
"""

import math
from contextlib import ExitStack

import numpy as np

N_NODES = 100000
N_EDGES = 3200000
D = 256
NCORES = 8
SUPER_W = 128
N_CHUNKS = 4
N_SUPERS = 99
GROUP_CAP = 1024

_PROGRAM_CACHE = {}


def _pack_dests(deg4, n_supers, cap):
    """LPT-pack dests (rows of deg4 [n_dests, 4]) into n_supers bins of
    <=128 dests, keeping every bin's per-chunk load <= cap when
    possible. Returns (sup_of, pos_of)."""
    n_dests = deg4.shape[0]
    loads = np.zeros((n_supers, 4), np.int64)
    slots = np.zeros(n_supers, np.int64)
    sup_of = np.empty(n_dests, np.int64)
    pos_of = np.empty(n_dests, np.int64)
    order = np.argsort(-deg4.sum(axis=1), kind="stable")
    for d in order:
        proj = loads + deg4[d]
        pmax = proj.max(axis=1)
        open_ = slots < 128
        feas = open_ & np.all(proj <= cap, axis=1)
        pool = feas if feas.any() else open_
        pmax_masked = np.where(pool, pmax, 1 << 60)
        b = int(np.argmin(pmax_masked))
        sup_of[d] = b
        pos_of[d] = slots[b]
        slots[b] += 1
        loads[b] += deg4[d]
    return sup_of, pos_of


def _plan(edge_row, edge_col, n_nodes, ncores, super_w, n_chunks, n_supers):
    rows_per_core = n_nodes // ncores
    chunk_sz = n_nodes // n_chunks

    core = edge_row // rows_per_core
    d_local = edge_row - core * rows_per_core
    chunk = edge_col // chunk_sz

    sup = np.empty(len(edge_row), np.int64)
    row_local = np.empty(len(edge_row), np.int64)
    devrow = np.empty((ncores, rows_per_core), np.int64)
    for k in range(ncores):
        sel = core == k
        dk = d_local[sel]
        ck = chunk[sel]
        deg4 = np.zeros((rows_per_core, n_chunks), np.int64)
        np.add.at(deg4, (dk, ck), 1)
        sup_of, pos_of = _pack_dests(deg4, n_supers, GROUP_CAP)
        sup[sel] = sup_of[dk]
        row_local[sel] = pos_of[dk]
        devrow[k] = sup_of * super_w + pos_of

    gid = sup * n_chunks + chunk
    n_groups = n_supers * n_chunks
    counts = np.zeros((ncores, n_groups), np.int64)
    np.add.at(counts, (core, gid), 1)
    caps = counts.max(axis=0)
    caps = np.maximum(((caps + 127) // 128) * 128, 128)
    return caps, core, row_local, sup, chunk, gid, devrow, chunk_sz


def _pack_core(k, caps, core, row_local, sup, chunk, gid, edge_col, edge_val,
               chunk_sz):
    n_groups = len(caps)
    sel = np.flatnonzero(core == k)
    g = gid[sel]
    order = np.argsort(g, kind="stable")
    sel = sel[order]
    g = g[order]

    cap_off = np.zeros(n_groups + 1, np.int64)
    np.cumsum(caps, out=cap_off[1:])

    grp_start = np.searchsorted(g, np.arange(n_groups))
    grp_end = np.searchsorted(g, np.arange(n_groups) + 1)
    assert np.all(grp_end - grp_start <= caps), "group cap overflow"
    rank = np.arange(len(g)) - grp_start[g]
    pos = cap_off[g] + rank

    tot = int(cap_off[-1])
    lc = np.zeros(tot, np.int16)
    rl = np.zeros(tot, np.float16)
    vv = np.zeros(tot, np.float16)
    lc[pos] = (edge_col[sel] - chunk[sel] * chunk_sz).astype(np.int16)
    rl[pos] = row_local[sel].astype(np.float16)
    vv[pos] = edge_val[sel].astype(np.float16)

    idx_planes = []
    for gi in range(n_groups):
        a, b = int(cap_off[gi]), int(cap_off[gi + 1])
        cap = b - a
        w16 = lc[a:b].reshape(cap // 16, 16).T
        idx_planes.append(np.tile(w16, (8, 1)))
    idx_plane = np.ascontiguousarray(np.concatenate(idx_planes, axis=1))
    rows = np.ascontiguousarray(rl.reshape(tot // 128, 128).T)
    vals = np.ascontiguousarray(vv.reshape(tot // 128, 128).T)
    return idx_plane, rows, vals


def _build_program(caps, n_nodes, super_w, n_supers, n_chunks, chunk_sz):
    import concourse.bacc as bacc
    import concourse.mybir as mybir
    import concourse.tile as tile
    from concourse import masks

    fp16 = mybir.dt.float16
    fp32 = mybir.dt.float32
    int16 = mybir.dt.int16
    n_groups = len(caps)
    rows_pad = n_supers * super_w
    nbs = caps // 128
    nb_max = int(nbs.max())
    nbtot = int(nbs.sum())

    idx_off = np.zeros(n_groups, np.int64)
    bat_off = np.zeros(n_groups, np.int64)
    o = 0
    b = 0
    for gi in range(n_groups):
        idx_off[gi] = o
        o += int(caps[gi]) // 16
        bat_off[gi] = b
        b += int(nbs[gi])
    tot_idx_cols = o

    nc = bacc.Bacc("TRN2", target_bir_lowering=False, num_swdge_queues=4,
                   dynamic_dma_scratch_size=49152)
    X16 = nc.dram_tensor("x16", [n_nodes, D], fp16, kind="ExternalInput")
    IDX = nc.dram_tensor("idxp", [128, tot_idx_cols], int16,
                         kind="ExternalInput")
    ROWS = nc.dram_tensor("rows", [128, nbtot], fp16, kind="ExternalInput")
    VALS = nc.dram_tensor("vals", [128, nbtot], fp16, kind="ExternalInput")
    VALS32 = nc.dram_tensor("vals32", [128, nbtot], fp32,
                            kind="ExternalInput")
    IOTA = nc.dram_tensor("iota", [128, nb_max, super_w], fp16,
                          kind="ExternalInput")
    WT = nc.dram_tensor("wt", [D, D], fp16, kind="ExternalInput")
    OUT = nc.dram_tensor("out", [rows_pad, D], fp32, kind="ExternalOutput")

    with tile.TileContext(nc) as tc, ExitStack() as ctx:
        const_pool = ctx.enter_context(tc.tile_pool(name="const", bufs=1))
        msgs_pool = ctx.enter_context(tc.tile_pool(name="msgs", bufs=10))
        o_pool = ctx.enter_context(tc.tile_pool(name="onehot", bufs=8))
        h_pool = ctx.enter_context(tc.tile_pool(name="h", bufs=2))
        out_pool = ctx.enter_context(tc.tile_pool(name="outp", bufs=3))
        psum_h_pool = ctx.enter_context(
            tc.tile_pool(name="psum_h", bufs=2, space="PSUM"))
        psum_t_pool = ctx.enter_context(
            tc.tile_pool(name="psum_t", bufs=2, space="PSUM"))
        psum_o_pool = ctx.enter_context(
            tc.tile_pool(name="psum_o", bufs=2, space="PSUM"))

        idx_t = const_pool.tile([128, tot_idx_cols], int16)
        nc.sync.dma_start(idx_t[:], IDX[:])
        rows_t = const_pool.tile([128, nbtot], fp16)
        nc.sync.dma_start(rows_t[:], ROWS[:])
        vals_t = const_pool.tile([128, nbtot], fp16)
        nc.sync.dma_start(vals_t[:], VALS[:])
        vals32_t = const_pool.tile([128, nbtot], fp32)
        nc.sync.dma_start(vals32_t[:], VALS32[:])
        iota_t = const_pool.tile([128, nb_max, super_w], fp16)
        nc.sync.dma_start(iota_t[:], IOTA[:])
        wt_t = const_pool.tile([128, 2, D], fp16)
        nc.sync.dma_start(wt_t[:, 0, :], WT[0:128, :])
        nc.sync.dma_start(wt_t[:, 1, :], WT[128:256, :])
        ident = const_pool.tile([128, 128], fp16)
        masks.make_identity(nc, ident[:])

        for s in range(n_supers):
            pH = psum_h_pool.tile([128, D], fp32, tag="ph")
            first = True
            for c in range(n_chunks):
                gi = s * n_chunks + c
                cap = int(caps[gi])
                nb = int(nbs[gi])
                bo = int(bat_off[gi])
                mt = msgs_pool.tile([128, nb, D], fp16, tag="msgs")
                nc.gpsimd.dma_gather(
                    mt[:],
                    X16[c * chunk_sz:(c + 1) * chunk_sz, :],
                    idx_t[:, int(idx_off[gi]):int(idx_off[gi]) + cap // 16],
                    cap,
                    cap,
                    D,
                    elem_step=D,
                    single_packet=(cap <= 1024),
                    queue_num=gi % 4,
                )
                oh = o_pool.tile([128, nb, super_w], fp16, tag="oh")
                nc.vector.tensor_tensor(
                    out=oh[:],
                    in0=iota_t[:, 0:nb, :],
                    in1=rows_t[:, bo:bo + nb, None].to_broadcast(
                        [128, nb, super_w]),
                    op=mybir.AluOpType.is_equal,
                )
                if gi % 2 == 0:
                    nc.vector.tensor_tensor(
                        out=oh[:],
                        in0=oh[:],
                        in1=vals_t[:, bo:bo + nb, None].to_broadcast(
                            [128, nb, super_w]),
                        op=mybir.AluOpType.mult,
                    )
                else:
                    for j in range(nb):
                        nc.scalar.mul(oh[:, j, :], oh[:, j, :],
                                      vals32_t[:, bo + j:bo + j + 1])
                for j in range(nb):
                    last = (c == n_chunks - 1) and (j == nb - 1)
                    nc.tensor.matmul(pH[:], oh[:, j, :], mt[:, j, :],
                                     start=first, stop=last)
                    first = False

            h = h_pool.tile([128, D], fp16, tag="h")
            nc.scalar.copy(h[:], pH[:])
            pT = psum_t_pool.tile([128, 2, 128], fp16, tag="pt")
            nc.tensor.transpose(pT[:, 0, :], h[:, 0:128], ident[:])
            nc.tensor.transpose(pT[:, 1, :], h[:, 128:256], ident[:])
            hT = h_pool.tile([128, 2, 128], fp16, tag="ht")
            nc.scalar.copy(hT[:, 0, :], pT[:, 0, :])
            nc.scalar.copy(hT[:, 1, :], pT[:, 1, :])
            po = psum_o_pool.tile([128, D], fp32, tag="po")
            nc.tensor.matmul(po[:], hT[:, 0, :], wt_t[:, 0, :],
                             start=True, stop=False)
            nc.tensor.matmul(po[:], hT[:, 1, :], wt_t[:, 1, :],
                             start=False, stop=True)
            ot = out_pool.tile([128, D], fp32, tag="ot")
            nc.scalar.copy(ot[:], po[:])
            nc.sync.dma_start(
                OUT[s * super_w:(s + 1) * super_w, :], ot[:])
    nc.finalize()
    return nc


def _prepare(X, edge_row, edge_col, edge_val, W):
    X = np.asarray(X)
    edge_row = np.asarray(edge_row)
    edge_col = np.asarray(edge_col)
    edge_val = np.asarray(edge_val)
    W = np.asarray(W)

    caps, core, row_local, sup, chunk, gid, devrow, chunk_sz = _plan(
        edge_row, edge_col, N_NODES, NCORES, SUPER_W, N_CHUNKS, N_SUPERS)

    key = (N_NODES, NCORES, SUPER_W, N_CHUNKS, tuple(caps.tolist()))
    if key not in _PROGRAM_CACHE:
        _PROGRAM_CACHE[key] = _build_program(
            caps, N_NODES, SUPER_W, N_SUPERS, N_CHUNKS, chunk_sz)
    nc = _PROGRAM_CACHE[key]

    nbs = caps // 128
    nb_max = int(nbs.max())
    X16 = np.ascontiguousarray(X.astype(np.float16))
    iota = np.tile(np.arange(SUPER_W, dtype=np.float16), (128, nb_max, 1))
    wt = np.ascontiguousarray(W.T.astype(np.float16))

    in_maps = []
    for k in range(NCORES):
        idx_plane, rows, vals = _pack_core(
            k, caps, core, row_local, sup, chunk, gid,
            edge_col, edge_val, chunk_sz)
        in_maps.append({"x16": X16, "idxp": idx_plane, "rows": rows,
                        "vals": vals, "vals32": vals.astype(np.float32),
                        "iota": iota, "wt": wt})
    return nc, in_maps, devrow


def _gather_out(res, b, devrow):
    rows_per_core = N_NODES // NCORES
    out = np.empty((N_NODES, D), np.float32)
    for k in range(NCORES):
        out[k * rows_per_core:(k + 1) * rows_per_core] = \
            res.results[k]["out"][devrow[k]]
    out += np.asarray(b).astype(np.float32)[None, :]
    return out


def kernel(X, edge_row, edge_col, edge_val, W, b):
    from concourse.bass_utils import run_bass_kernel_spmd

    nc, in_maps, devrow = _prepare(X, edge_row, edge_col, edge_val, W)
    res = run_bass_kernel_spmd(nc, in_maps, core_ids=list(range(NCORES)))
    return _gather_out(res, b, devrow)


def run_traced(X, edge_row, edge_col, edge_val, W, b, tmpdir=None):
    from concourse.bass_utils import run_bass_kernel_spmd

    nc, in_maps, _ = _prepare(X, edge_row, edge_col, edge_val, W)
    return run_bass_kernel_spmd(nc, in_maps, core_ids=list(range(NCORES)),
                                trace=True, tmpdir=tmpdir)
